# revision 13
# baseline (speedup 1.0000x reference)
"""GQA attention (RoPE, causal) for nn_Attention_43293270343986 on 8 TRN2 cores.

Sharding: tensor-parallel over the 8 KV head groups. Core c owns kv head c and
its 4 query heads (Wq/Wk/Wv column-sharded, Wo row-sharded); the host sums the
8 partial outputs (the all-reduce equivalent).

Device-side layout is feature-major ("transposed"): the host passes
hidden_states pre-transposed as hT [D, T] so every matmul contracts over the
partition dimension with zero on-device transposes of activations:

  qT = Wq_c.T @ hT   (per 2-head pair, PSUM accum over 16 K-chunks of 128)
  kT likewise, with Wk columns duplicated so kT lands twice (partitions 0-63
  and 64-127) letting even/odd heads' score matmuls use disjoint PE row groups
  vT -> V (token-major) via PE transpose; a ones-column is appended so the
  P@V matmul also produces the softmax denominator as output row 64
  ST[k,q] = K @ Q^T per head  -> exp (no max subtraction: scores ~ N(0,1),
  fp32 exp cannot overflow)   -> multiplicative 0/1 causal mask on the 4
  diagonal k-blocks only; fully-masked k-blocks are skipped entirely
  O~ = [V|1].T @ P^T  -> rows 0-63 unnormalized O^T, row 64 the denominator
  normalize via reciprocal + gpsimd partition_broadcast
  Y^T = Wo_c.T-chunks @ O^T   (PSUM accum over the 2 128-feature chunks)

Output is Y^T [D, T] fp32 per core; host sums over cores and transposes back.
"""

from contextlib import ExitStack

import numpy as np

import concourse.bacc as bacc
import concourse.mybir as mybir
import concourse.tile as tile
from concourse.bass_utils import run_bass_kernel_spmd
from concourse.masks import make_identity

B, S, D = 2, 2048, 2048
HQ, HKV, DH = 32, 8, 64
ROPE_THETA = 10000.0

N_CORES = 8
T = B * S               # 4096 tokens (batch-major concat)
NQH = HQ // HKV         # 4 query heads per core
QC = NQH * DH           # 256 q-projection cols per core
TT = 512                # token tile (matmul moving dim)
KC = D // 128           # 16 contraction chunks for the projections
KH = KC // 2            # ht is loaded in two halves of 8 chunks
NTAU = T // TT          # 8 token tiles
QW = S // TT            # 4 query windows per batch
NKB = S // 128          # 16 key blocks per batch

# "f32" (exact), "f32r" (full-speed fp32 replicated mode), "bf16"
MM_MODE = "bf16"

_F32 = mybir.dt.float32
_F32R = mybir.dt.float32r
_BF16 = mybir.dt.bfloat16


def _np_dt(mode):
    import ml_dtypes
    return np.dtype(ml_dtypes.bfloat16) if mode == "bf16" else np.dtype(np.float32)


def _store_dt(mode):
    return _BF16 if mode == "bf16" else _F32


def build_nc(mode=None):
    mode = mode or MM_MODE
    sdt = _store_dt(mode)

    nc = bacc.Bacc("TRN2", target_bir_lowering=False, debug=False,
                   num_devices=N_CORES)

    ht_d = nc.dram_tensor("ht", [D, T], sdt, kind="ExternalInput")
    wq_d = nc.dram_tensor("wq", [D, QC], sdt, kind="ExternalInput")
    wk2_d = nc.dram_tensor("wk2", [D, 128], sdt, kind="ExternalInput")
    wv_d = nc.dram_tensor("wv", [D, DH], sdt, kind="ExternalInput")
    wo_d = nc.dram_tensor("wo", [QC, D], sdt, kind="ExternalInput")
    cos2_d = nc.dram_tensor("cos2", [128, S], _F32, kind="ExternalInput")
    sin2_d = nc.dram_tensor("sin2", [128, S], _F32, kind="ExternalInput")
    r2_d = nc.dram_tensor("r2", [128, 128], sdt, kind="ExternalInput")
    masks_d = nc.dram_tensor("masks", [128, QW, TT], sdt, kind="ExternalInput")
    yt_d = nc.dram_tensor("yt", [D, T], _F32, kind="ExternalOutput")

    ht_r = ht_d[:].rearrange("(ko p) t -> p ko t", p=128)

    def mm(ps, lhsT, rhs, start, stop):
        if mode == "f32r":
            lhsT = lhsT.bitcast(_F32R)
            rhs = rhs.bitcast(_F32R)
        nc.tensor.matmul(ps, lhsT, rhs, start=start, stop=stop)

    with tile.TileContext(nc) as tc:
        es_o = ExitStack()
        es_qkv = ExitStack()
        op = es_o.enter_context(tc.tile_pool(name="op", bufs=1))
        qkvp = es_qkv.enter_context(tc.tile_pool(name="qkvp", bufs=1))
        qT_all = qkvp.tile([128, 2, T], sdt)     # head pairs on partitions
        kT_dup = qkvp.tile([128, T], sdt)        # kT duplicated on both halves
        v_all = qkvp.tile([128, T // 128, DH + 1], sdt)

        # ---------------- Phase A: QKV projections + RoPE ----------------
        with (
            tc.tile_pool(name="aconsts", bufs=1) as aconsts,
            tc.tile_pool(name="htp", bufs=3) as htp,
            tc.tile_pool(name="atmp", bufs=2) as atmp,
            tc.tile_pool(name="apsum", bufs=6, space="PSUM") as apsum,
            tc.tile_pool(name="tpsum", bufs=2, space="PSUM") as tpsum,
        ):
            wq_sb = aconsts.tile([128, KC, QC], sdt)
            wk2_sb = aconsts.tile([128, KC, 128], sdt)
            wv_sb = aconsts.tile([128, KC, DH], sdt)
            cos2_sb = aconsts.tile([128, S], _F32)
            sin2_sb = aconsts.tile([128, S], _F32)
            r2_sb = aconsts.tile([128, 128], sdt)
            id64 = aconsts.tile([64, 64], sdt)
            nc.sync.dma_start(wq_sb, wq_d[:].rearrange("(ko p) c -> p ko c", p=128))
            nc.sync.dma_start(wk2_sb, wk2_d[:].rearrange("(ko p) c -> p ko c", p=128))
            nc.sync.dma_start(wv_sb, wv_d[:].rearrange("(ko p) c -> p ko c", p=128))
            nc.sync.dma_start(cos2_sb, cos2_d[:])
            nc.sync.dma_start(sin2_sb, sin2_d[:])
            nc.sync.dma_start(r2_sb, r2_d[:])
            make_identity(nc, id64)
            nc.any.memset(v_all[:, :, DH:DH + 1], 1.0)

            for tau in range(NTAU):
                tok = tau * TT
                pos = (tau % QW) * TT
                cs = cos2_sb[:, pos:pos + TT]
                sn = sin2_sb[:, pos:pos + TT]

                hts = []
                for half in range(2):
                    ht = htp.tile([128, KH, TT], sdt, tag="ht")
                    nc.sync.dma_start(
                        ht, ht_r[:, half * KH:(half + 1) * KH, tok:tok + TT])
                    hts.append(ht)

                def proj(w_sb, wcols, np_, accum_ps=None):
                    ps = accum_ps or apsum.tile([np_, TT], _F32, tag="pa")
                    for k in range(KC):
                        mm(ps, w_sb[:, k, wcols], hts[k // KH][:, k % KH, :],
                           start=(k == 0), stop=(k == KC - 1))
                    return ps

                def rope_pair(w_sb, wcols, dst):
                    ps = proj(w_sb, wcols, 128)
                    raw = atmp.tile([128, TT], sdt, tag="raw")
                    nc.vector.tensor_copy(raw, ps)
                    psr = apsum.tile([128, TT], _F32, tag="pa")
                    mm(psr, r2_sb, raw, start=True, stop=True)
                    nc.vector.tensor_mul(dst, ps, cs)
                    tsin = atmp.tile([128, TT], sdt, tag="tsin")
                    nc.vector.tensor_mul(tsin, psr, sn)
                    nc.vector.tensor_add(dst, dst, tsin)

                for j in range(2):
                    rope_pair(wq_sb, slice(j * 128, (j + 1) * 128),
                              qT_all[:, j, tok:tok + TT])
                rope_pair(wk2_sb, slice(0, 128), kT_dup[:, tok:tok + TT])

                # V: project feature-major then PE-transpose to token-major
                psv = proj(wv_sb, slice(0, DH), 64)
                vT = atmp.tile([64, TT], sdt, tag="vT")
                nc.vector.tensor_copy(vT, psv)
                for c in range(TT // 128):
                    pvt = tpsum.tile([128, DH], sdt, tag="pvt")
                    nc.tensor.transpose(pvt, vT[:, c * 128:(c + 1) * 128], id64)
                    nc.vector.tensor_copy(
                        v_all[:, tau * (TT // 128) + c, 0:DH], pvt)

        # ------- Phase B: attention + fused output projection -------
        # Heads are processed in pairs (even head on PE rows 0-63, odd on
        # 64-127). Each kb step writes both heads' scores into one 2-bank
        # PSUM tile so a single exp covers the pair. After the window's 4
        # heads are normalized into oT_all, the Wo matmuls for that token
        # window run immediately so the output DMA streams during attention.
        oT_all = op.tile([128, 2, T], sdt)
        with (
            tc.tile_pool(name="bconsts", bufs=1) as bconsts,
            tc.tile_pool(name="ptp", bufs=3) as ptp,
            tc.tile_pool(name="btmp", bufs=3) as btmp,
            tc.tile_pool(name="yp", bufs=4) as yp,
            tc.tile_pool(name="spsum", bufs=2, space="PSUM") as spsum,
            tc.tile_pool(name="opsum", bufs=2, space="PSUM") as opsum,
            tc.tile_pool(name="cpsum", bufs=2, space="PSUM") as cpsum,
        ):
            masks_sb = bconsts.tile([128, QW, TT], sdt)
            wo_sb = bconsts.tile([128, 2, D], sdt)
            nc.sync.dma_start(masks_sb, masks_d[:])
            nc.sync.dma_start(wo_sb, wo_d[:].rearrange("(c p) n -> p c n", p=128))

            for b in range(B):
                for qw in range(QW):
                    tok0 = b * S + qw * TT
                    nkb = (TT // 128) * (qw + 1)
                    for hp in range(2):          # head pair = (2*hp, 2*hp+1)
                        pso = [opsum.tile([DH + 1, TT], _F32, tag="pso",
                                          name=f"pso{_i}")
                               for _i in range(2)]
                        for kb in range(nkb):
                            kt0 = b * S + kb * 128
                            r = kb - (TT // 128) * qw
                            w = 128 * r if r >= 0 else 0   # fully-masked cols
                            pss = spsum.tile([128, 2, TT], _F32, tag="pss")
                            pt = ptp.tile([128, 2, TT], sdt, tag="pt")
                            for i in range(2):
                                off = i * 64
                                mm(pss[:, i, w:],
                                   kT_dup[off:off + 64, kt0:kt0 + 128],
                                   qT_all[off:off + 64, hp, tok0 + w:tok0 + TT],
                                   start=True, stop=True)
                            nc.scalar.activation(
                                pt[:, :, w:], pss[:, :, w:],
                                mybir.ActivationFunctionType.Exp,
                                scale=float(1.0 / np.sqrt(DH)))
                            if r >= 0:
                                # only the [128,128] diagonal block is partial
                                for i in range(2):
                                    nc.vector.tensor_mul(
                                        pt[:, i, w:w + 128], pt[:, i, w:w + 128],
                                        masks_sb[:, r, w:w + 128])
                            for i in range(2):
                                # cols < w are fully masked: skip them in the
                                # accumulation (earlier kbs covered them)
                                mm(pso[i][:, w:], v_all[:, b * NKB + kb, :],
                                   pt[:, i, w:],
                                   start=(kb == 0), stop=(kb == nkb - 1))
                        for i in range(2):
                            rec = btmp.tile([DH + 1, TT], _F32, tag="rec")
                            nc.vector.reciprocal(rec[DH:DH + 1, :],
                                                 pso[i][DH:DH + 1, :])
                            # partition_broadcast mis-reads base!=0 sources on
                            # HW; hop the row to partition 0 via DMA first
                            rec0 = btmp.tile([1, TT], _F32, tag="rec0")
                            nc.sync.dma_start(rec0, rec[DH:DH + 1, :])
                            bc = btmp.tile([64, TT], _F32, tag="bc")
                            nc.gpsimd.partition_broadcast(bc, rec0[0:1, :])
                            if i == 0:
                                nc.vector.tensor_mul(
                                    oT_all[0:64, hp, tok0:tok0 + TT],
                                    pso[i][0:DH, :], bc)
                            else:
                                onrm = btmp.tile([64, TT], sdt, tag="onrm")
                                nc.vector.tensor_mul(onrm, pso[i][0:DH, :], bc)
                                nc.sync.dma_start(
                                    oT_all[64:128, hp, tok0:tok0 + TT], onrm)
                    # fused output projection for this token window
                    for m in range(D // 128):
                        psy = cpsum.tile([128, TT], _F32, tag="psy")
                        for ch in range(2):
                            mm(psy, wo_sb[:, ch, m * 128:(m + 1) * 128],
                               oT_all[:, ch, tok0:tok0 + TT],
                               start=(ch == 0), stop=(ch == 1))
                        ysb = yp.tile([128, TT], _F32, tag="ysb")
                        if m % 2 == 0:
                            nc.vector.tensor_copy(ysb, psy)
                        else:
                            nc.scalar.copy(ysb, psy)
                        nc.sync.dma_start(
                            yt_d[m * 128:(m + 1) * 128, tok0:tok0 + TT], ysb)

        es_qkv.close()
        es_o.close()

    nc.compile()
    return nc


def _rope_tables():
    inv_freq = 1.0 / (ROPE_THETA ** (np.arange(0, DH, 2, dtype=np.float64) / DH))
    t = np.arange(S, dtype=np.float64)
    freqs = np.outer(t, inv_freq)
    emb = np.concatenate([freqs, freqs], axis=-1)      # [S, DH]
    return (np.cos(emb).astype(np.float32), np.sin(emb).astype(np.float32))


def _rot_matrix():
    # (R64.T @ qT)[d] == rotate_half(q)[d]: rot[d] = -q[d+32] (d<32), q[d-32] (d>=32)
    r = np.zeros((64, 64), np.float32)
    for d in range(32):
        r[d + 32, d] = -1.0
        r[d, d + 32] = 1.0
    r2 = np.zeros((128, 128), np.float32)
    r2[0:64, 0:64] = r
    r2[64:128, 64:128] = r
    return r2


def make_in_maps(hidden_states, Wq, Wk, Wv, Wo, mode=None):
    mode = mode or MM_MODE
    ndt = _np_dt(mode)
    hT = np.ascontiguousarray(
        np.asarray(hidden_states, np.float32).reshape(T, D).T).astype(ndt)
    cos, sin = _rope_tables()                          # [S, DH]
    cos2 = np.ascontiguousarray(
        np.concatenate([cos.T, cos.T], axis=0)).astype(np.float32)  # [128, S]
    sin2 = np.ascontiguousarray(
        np.concatenate([sin.T, sin.T], axis=0)).astype(np.float32)
    r2 = _rot_matrix().astype(ndt)

    kk = np.arange(128)[:, None, None]
    rr = np.arange(QW)[None, :, None]
    qq = np.arange(TT)[None, None, :]
    masks = np.ascontiguousarray(
        (128 * rr + kk <= qq).astype(ndt))             # [128, QW, TT]

    Wq = np.asarray(Wq, np.float32)
    Wk = np.asarray(Wk, np.float32)
    Wv = np.asarray(Wv, np.float32)
    Wo = np.asarray(Wo, np.float32)

    in_maps = []
    for c in range(N_CORES):
        wq_c = np.ascontiguousarray(Wq[:, c * QC:(c + 1) * QC]).astype(ndt)
        wk_c = Wk[:, c * DH:(c + 1) * DH]
        wk2_c = np.ascontiguousarray(
            np.concatenate([wk_c, wk_c], axis=1)).astype(ndt)
        wv_c = np.ascontiguousarray(Wv[:, c * DH:(c + 1) * DH]).astype(ndt)
        wo_c = np.ascontiguousarray(Wo[c * QC:(c + 1) * QC, :]).astype(ndt)
        in_maps.append({
            "ht": hT, "wq": wq_c, "wk2": wk2_c, "wv": wv_c, "wo": wo_c,
            "cos2": cos2, "sin2": sin2, "r2": r2, "masks": masks,
        })
    return in_maps


def postprocess(results):
    acc = np.zeros((D, T), np.float32)
    for res in results:
        acc += res["yt"]
    return np.ascontiguousarray(acc.T).reshape(B, S, D)


def kernel(hidden_states, Wq, Wk, Wv, Wo):
    nc = build_nc()
    in_maps = make_in_maps(hidden_states, Wq, Wk, Wv, Wo)
    res = run_bass_kernel_spmd(nc, in_maps, core_ids=list(range(N_CORES)))
    return postprocess(res.results)


# revision 14
# speedup vs baseline: 2.7380x; 2.7380x over previous
"""GQA attention (RoPE, causal) for nn_Attention_43293270343986 on 8 TRN2 cores.

Sharding: tensor-parallel over the 8 KV head groups. Core c owns kv head c and
its 4 query heads (Wq/Wk/Wv column-sharded, Wo row-sharded); the host sums the
8 partial outputs (the all-reduce equivalent).

Device-side layout is feature-major ("transposed"): the host passes
hidden_states pre-transposed as hT [D, T] so every matmul contracts over the
partition dimension with zero on-device transposes of activations:

  qT = Wq_c.T @ hT   (per 2-head pair, PSUM accum over 16 K-chunks of 128)
  kT likewise, with Wk columns duplicated so kT lands twice (partitions 0-63
  and 64-127) letting even/odd heads' score matmuls use disjoint PE row groups
  vT -> V (token-major) via PE transpose; a ones-column is appended so the
  P@V matmul also produces the softmax denominator as output row 64
  ST[k,q] = K @ Q^T per head  -> exp (no max subtraction: scores ~ N(0,1),
  fp32 exp cannot overflow)   -> multiplicative 0/1 causal mask on the 4
  diagonal k-blocks only; fully-masked k-blocks are skipped entirely
  O~ = [V|1].T @ P^T  -> rows 0-63 unnormalized O^T, row 64 the denominator
  normalize via reciprocal + gpsimd partition_broadcast
  Y^T = Wo_c.T-chunks @ O^T   (PSUM accum over the 2 128-feature chunks)

Output is Y^T [D, T] fp32 per core; host sums over cores and transposes back.
"""

from contextlib import ExitStack

import numpy as np

import concourse.bacc as bacc
import concourse.mybir as mybir
import concourse.tile as tile
from concourse.bass_utils import run_bass_kernel_spmd
from concourse.masks import make_identity

B, S, D = 2, 2048, 2048
HQ, HKV, DH = 32, 8, 64
ROPE_THETA = 10000.0

N_CORES = 8
T = B * S               # 4096 tokens (batch-major concat)
NQH = HQ // HKV         # 4 query heads per core
QC = NQH * DH           # 256 q-projection cols per core
TT = 512                # token tile (matmul moving dim)
KC = D // 128           # 16 contraction chunks for the projections
KH = KC // 2            # ht is loaded in two halves of 8 chunks
NTAU = T // TT          # 8 token tiles
QW = S // TT            # 4 query windows per batch
NKB = S // 128          # 16 key blocks per batch

# "f32" (exact, 4 cyc/row), "bf16"/"f16" (1 cyc/row); f16 has 8x finer
# mantissa than bf16 and every tensor here fits fp16 range
MM_MODE = "f16"

_F32 = mybir.dt.float32
_F32R = mybir.dt.float32r
_BF16 = mybir.dt.bfloat16
_F16 = mybir.dt.float16


def _np_dt(mode):
    import ml_dtypes
    if mode == "bf16":
        return np.dtype(ml_dtypes.bfloat16)
    if mode == "f16":
        return np.dtype(np.float16)
    return np.dtype(np.float32)


def _store_dt(mode):
    return {"bf16": _BF16, "f16": _F16}.get(mode, _F32)


def build_nc(mode=None):
    mode = mode or MM_MODE
    sdt = _store_dt(mode)

    nc = bacc.Bacc("TRN2", target_bir_lowering=False, debug=False,
                   num_devices=N_CORES)

    ht_d = nc.dram_tensor("ht", [D, T], sdt, kind="ExternalInput")
    wq_d = nc.dram_tensor("wq", [D, QC], sdt, kind="ExternalInput")
    wk2_d = nc.dram_tensor("wk2", [D, 128], sdt, kind="ExternalInput")
    wv_d = nc.dram_tensor("wv", [D, DH], sdt, kind="ExternalInput")
    wo_d = nc.dram_tensor("wo", [QC, D], sdt, kind="ExternalInput")
    cos2_d = nc.dram_tensor("cos2", [128, S], _F32, kind="ExternalInput")
    sin2_d = nc.dram_tensor("sin2", [128, S], _F32, kind="ExternalInput")
    r2_d = nc.dram_tensor("r2", [128, 128], sdt, kind="ExternalInput")
    masks_d = nc.dram_tensor("masks", [128, QW, TT], sdt, kind="ExternalInput")
    yt_d = nc.dram_tensor("yt", [D, T], _F32, kind="ExternalOutput")

    ht_r = ht_d[:].rearrange("(ko p) t -> p ko t", p=128)

    def mm(ps, lhsT, rhs, start, stop):
        if mode == "f32r":
            lhsT = lhsT.bitcast(_F32R)
            rhs = rhs.bitcast(_F32R)
        nc.tensor.matmul(ps, lhsT, rhs, start=start, stop=stop)

    with tile.TileContext(nc) as tc:
        es_o = ExitStack()
        es_qkv = ExitStack()
        op = es_o.enter_context(tc.tile_pool(name="op", bufs=1))
        qkvp = es_qkv.enter_context(tc.tile_pool(name="qkvp", bufs=1))
        qT_all = qkvp.tile([128, 2, T], sdt)     # head pairs on partitions
        kT_dup = qkvp.tile([128, T], sdt)        # kT duplicated on both halves
        v_all = qkvp.tile([128, T // 128, DH + 1], sdt)

        # ---------------- Phase A: QKV projections + RoPE ----------------
        with (
            tc.tile_pool(name="aconsts", bufs=1) as aconsts,
            tc.tile_pool(name="htp", bufs=3) as htp,
            tc.tile_pool(name="atmp", bufs=2) as atmp,
            tc.tile_pool(name="apsum", bufs=6, space="PSUM") as apsum,
            tc.tile_pool(name="tpsum", bufs=2, space="PSUM") as tpsum,
        ):
            wq_sb = aconsts.tile([128, KC, QC], sdt)
            wk2_sb = aconsts.tile([128, KC, 128], sdt)
            wv_sb = aconsts.tile([128, KC, DH], sdt)
            cos2_sb = aconsts.tile([128, S], _F32)
            sin2_sb = aconsts.tile([128, S], _F32)
            r2_sb = aconsts.tile([128, 128], sdt)
            id64 = aconsts.tile([64, 64], sdt)
            nc.sync.dma_start(wq_sb, wq_d[:].rearrange("(ko p) c -> p ko c", p=128))
            nc.sync.dma_start(wk2_sb, wk2_d[:].rearrange("(ko p) c -> p ko c", p=128))
            nc.sync.dma_start(wv_sb, wv_d[:].rearrange("(ko p) c -> p ko c", p=128))
            nc.sync.dma_start(cos2_sb, cos2_d[:])
            nc.sync.dma_start(sin2_sb, sin2_d[:])
            nc.sync.dma_start(r2_sb, r2_d[:])
            make_identity(nc, id64)
            nc.any.memset(v_all[:, :, DH:DH + 1], 1.0)

            for tau in range(NTAU):
                tok = tau * TT
                pos = (tau % QW) * TT
                cs = cos2_sb[:, pos:pos + TT]
                sn = sin2_sb[:, pos:pos + TT]

                hts = []
                for half in range(2):
                    ht = htp.tile([128, KH, TT], sdt, tag="ht")
                    nc.sync.dma_start(
                        ht, ht_r[:, half * KH:(half + 1) * KH, tok:tok + TT])
                    hts.append(ht)

                def proj(w_sb, wcols, np_, accum_ps=None):
                    ps = accum_ps or apsum.tile([np_, TT], _F32, tag="pa")
                    for k in range(KC):
                        mm(ps, w_sb[:, k, wcols], hts[k // KH][:, k % KH, :],
                           start=(k == 0), stop=(k == KC - 1))
                    return ps

                def rope_pair(w_sb, wcols, dst):
                    ps = proj(w_sb, wcols, 128)
                    raw = atmp.tile([128, TT], sdt, tag="raw")
                    nc.vector.tensor_copy(raw, ps)
                    psr = apsum.tile([128, TT], _F32, tag="pa")
                    mm(psr, r2_sb, raw, start=True, stop=True)
                    nc.vector.tensor_mul(dst, ps, cs)
                    tsin = atmp.tile([128, TT], sdt, tag="tsin")
                    nc.vector.tensor_mul(tsin, psr, sn)
                    nc.vector.tensor_add(dst, dst, tsin)

                for j in range(2):
                    rope_pair(wq_sb, slice(j * 128, (j + 1) * 128),
                              qT_all[:, j, tok:tok + TT])
                rope_pair(wk2_sb, slice(0, 128), kT_dup[:, tok:tok + TT])

                # V: project feature-major then PE-transpose to token-major
                psv = proj(wv_sb, slice(0, DH), 64)
                vT = atmp.tile([64, TT], sdt, tag="vT")
                nc.vector.tensor_copy(vT, psv)
                for c in range(TT // 128):
                    pvt = tpsum.tile([128, DH], sdt, tag="pvt")
                    nc.tensor.transpose(pvt, vT[:, c * 128:(c + 1) * 128], id64)
                    nc.vector.tensor_copy(
                        v_all[:, tau * (TT // 128) + c, 0:DH], pvt)

        # ------- Phase B: attention + fused output projection -------
        # Heads are processed in pairs (even head on PE rows 0-63, odd on
        # 64-127). Each kb step writes both heads' scores into one 2-bank
        # PSUM tile so a single exp covers the pair. After the window's 4
        # heads are normalized into oT_all, the Wo matmuls for that token
        # window run immediately so the output DMA streams during attention.
        oT_all = op.tile([128, 2, T], sdt)
        with (
            tc.tile_pool(name="bconsts", bufs=1) as bconsts,
            tc.tile_pool(name="ptp", bufs=3) as ptp,
            tc.tile_pool(name="btmp", bufs=3) as btmp,
            tc.tile_pool(name="yp", bufs=4) as yp,
            tc.tile_pool(name="spsum", bufs=2, space="PSUM") as spsum,
            tc.tile_pool(name="opsum", bufs=2, space="PSUM") as opsum,
            tc.tile_pool(name="cpsum", bufs=2, space="PSUM") as cpsum,
        ):
            masks_sb = bconsts.tile([128, QW, TT], sdt)
            wo_sb = bconsts.tile([128, 2, D], sdt)
            nc.sync.dma_start(masks_sb, masks_d[:])
            nc.sync.dma_start(wo_sb, wo_d[:].rearrange("(c p) n -> p c n", p=128))

            for b in range(B):
                for qw in range(QW):
                    tok0 = b * S + qw * TT
                    nkb = (TT // 128) * (qw + 1)
                    for hp in range(2):          # head pair = (2*hp, 2*hp+1)
                        pso = [opsum.tile([DH + 1, TT], _F32, tag="pso",
                                          name=f"pso{_i}")
                               for _i in range(2)]
                        for kb in range(nkb):
                            kt0 = b * S + kb * 128
                            r = kb - (TT // 128) * qw
                            w = 128 * r if r >= 0 else 0   # fully-masked cols
                            pss = spsum.tile([128, 2, TT], _F32, tag="pss")
                            pt = ptp.tile([128, 2, TT], sdt, tag="pt")
                            for i in range(2):
                                off = i * 64
                                mm(pss[:, i, w:],
                                   kT_dup[off:off + 64, kt0:kt0 + 128],
                                   qT_all[off:off + 64, hp, tok0 + w:tok0 + TT],
                                   start=True, stop=True)
                            nc.scalar.activation(
                                pt[:, :, w:], pss[:, :, w:],
                                mybir.ActivationFunctionType.Exp,
                                scale=float(1.0 / np.sqrt(DH)))
                            if r >= 0:
                                # only the [128,128] diagonal block is partial
                                for i in range(2):
                                    nc.vector.tensor_mul(
                                        pt[:, i, w:w + 128], pt[:, i, w:w + 128],
                                        masks_sb[:, r, w:w + 128])
                            for i in range(2):
                                # cols < w are fully masked: skip them in the
                                # accumulation (earlier kbs covered them)
                                mm(pso[i][:, w:], v_all[:, b * NKB + kb, :],
                                   pt[:, i, w:],
                                   start=(kb == 0), stop=(kb == nkb - 1))
                        for i in range(2):
                            rec = btmp.tile([DH + 1, TT], _F32, tag="rec")
                            nc.vector.reciprocal(rec[DH:DH + 1, :],
                                                 pso[i][DH:DH + 1, :])
                            # partition_broadcast mis-reads base!=0 sources on
                            # HW; hop the row to partition 0 via DMA first
                            rec0 = btmp.tile([1, TT], _F32, tag="rec0")
                            nc.sync.dma_start(rec0, rec[DH:DH + 1, :])
                            bc = btmp.tile([64, TT], _F32, tag="bc")
                            nc.gpsimd.partition_broadcast(bc, rec0[0:1, :])
                            if i == 0:
                                nc.vector.tensor_mul(
                                    oT_all[0:64, hp, tok0:tok0 + TT],
                                    pso[i][0:DH, :], bc)
                            else:
                                onrm = btmp.tile([64, TT], sdt, tag="onrm")
                                nc.vector.tensor_mul(onrm, pso[i][0:DH, :], bc)
                                nc.sync.dma_start(
                                    oT_all[64:128, hp, tok0:tok0 + TT], onrm)
                    # fused output projection for this token window
                    for m in range(D // 128):
                        psy = cpsum.tile([128, TT], _F32, tag="psy")
                        for ch in range(2):
                            mm(psy, wo_sb[:, ch, m * 128:(m + 1) * 128],
                               oT_all[:, ch, tok0:tok0 + TT],
                               start=(ch == 0), stop=(ch == 1))
                        ysb = yp.tile([128, TT], _F32, tag="ysb")
                        if m % 2 == 0:
                            nc.vector.tensor_copy(ysb, psy)
                        else:
                            nc.scalar.copy(ysb, psy)
                        nc.sync.dma_start(
                            yt_d[m * 128:(m + 1) * 128, tok0:tok0 + TT], ysb)

        es_qkv.close()
        es_o.close()

    nc.compile()
    return nc


def _rope_tables():
    inv_freq = 1.0 / (ROPE_THETA ** (np.arange(0, DH, 2, dtype=np.float64) / DH))
    t = np.arange(S, dtype=np.float64)
    freqs = np.outer(t, inv_freq)
    emb = np.concatenate([freqs, freqs], axis=-1)      # [S, DH]
    return (np.cos(emb).astype(np.float32), np.sin(emb).astype(np.float32))


def _rot_matrix():
    # (R64.T @ qT)[d] == rotate_half(q)[d]: rot[d] = -q[d+32] (d<32), q[d-32] (d>=32)
    r = np.zeros((64, 64), np.float32)
    for d in range(32):
        r[d + 32, d] = -1.0
        r[d, d + 32] = 1.0
    r2 = np.zeros((128, 128), np.float32)
    r2[0:64, 0:64] = r
    r2[64:128, 64:128] = r
    return r2


def make_in_maps(hidden_states, Wq, Wk, Wv, Wo, mode=None):
    mode = mode or MM_MODE
    ndt = _np_dt(mode)
    hT = np.ascontiguousarray(
        np.asarray(hidden_states, np.float32).reshape(T, D).T).astype(ndt)
    cos, sin = _rope_tables()                          # [S, DH]
    cos2 = np.ascontiguousarray(
        np.concatenate([cos.T, cos.T], axis=0)).astype(np.float32)  # [128, S]
    sin2 = np.ascontiguousarray(
        np.concatenate([sin.T, sin.T], axis=0)).astype(np.float32)
    r2 = _rot_matrix().astype(ndt)

    kk = np.arange(128)[:, None, None]
    rr = np.arange(QW)[None, :, None]
    qq = np.arange(TT)[None, None, :]
    masks = np.ascontiguousarray(
        (128 * rr + kk <= qq).astype(ndt))             # [128, QW, TT]

    Wq = np.asarray(Wq, np.float32)
    Wk = np.asarray(Wk, np.float32)
    Wv = np.asarray(Wv, np.float32)
    Wo = np.asarray(Wo, np.float32)

    in_maps = []
    for c in range(N_CORES):
        wq_c = np.ascontiguousarray(Wq[:, c * QC:(c + 1) * QC]).astype(ndt)
        wk_c = Wk[:, c * DH:(c + 1) * DH]
        wk2_c = np.ascontiguousarray(
            np.concatenate([wk_c, wk_c], axis=1)).astype(ndt)
        wv_c = np.ascontiguousarray(Wv[:, c * DH:(c + 1) * DH]).astype(ndt)
        wo_c = np.ascontiguousarray(Wo[c * QC:(c + 1) * QC, :]).astype(ndt)
        in_maps.append({
            "ht": hT, "wq": wq_c, "wk2": wk2_c, "wv": wv_c, "wo": wo_c,
            "cos2": cos2, "sin2": sin2, "r2": r2, "masks": masks,
        })
    return in_maps


def postprocess(results):
    acc = np.zeros((D, T), np.float32)
    for res in results:
        acc += res["yt"]
    return np.ascontiguousarray(acc.T).reshape(B, S, D)


def kernel(hidden_states, Wq, Wk, Wv, Wo):
    nc = build_nc()
    in_maps = make_in_maps(hidden_states, Wq, Wk, Wv, Wo)
    res = run_bass_kernel_spmd(nc, in_maps, core_ids=list(range(N_CORES)))
    return postprocess(res.results)
